# revision 58
# baseline (speedup 1.0000x reference)
"""MAE ViT (PrawnMKPModel) forward pass on 8 TRN2 NeuronCores.

Strategy: pure data parallelism (8 samples/core). All ragged gather/pad/
scatter work is hoisted to host-side input prep by exploiting permutation
equivariance of attention: the decoder runs in *permuted* token order
(visible tokens first, masked after), so the device kernel is a fully
dense transformer. Host folds LN affines into adjacent matmul weights,
pre-scales Q by 1/sqrt(dh), and un-permutes the output at the end.

Device pipeline per core (8 samples):
  patch-embed matmul -> 6 encoder blocks (seq 50, pad 64/sample, D=384)
  -> dec-embed -> 4 decoder blocks (seq 196 = 2x98 tiles/sample, DD=256)
  -> pred head. Softmax is computed un-normalized (scores are small, no
  max-subtraction needed); the denominator Z comes for free from an
  appended ones-column on V, and 1/Z is applied during the batched AV
  eviction via a stride-0-broadcast multiply.

fp8 (e4m3) DoubleRow matmuls (0.5 cycles/row, 2 stacked k-tiles = 4x bf16
throughput) are used where a numpy error study showed they fit the 2e-2
rel-err budget: encoder QKV + out-proj (contract padded 384->512 with zero
weight rows; LN1 output written fp8 by folding ln(SXE) into the rsqrt Exp
bias) and the decoder AV (exp scores written fp8; V scaled by SVD which
cancels through the ones-column Z). Decoder QKV/FFN, patch embed, dec-embed
and pred head stay bf16: fp8 there adds 2e-2..4e-2 rel err (measured).

Engine assignment (busy-balance against the CoreSim cost model; GPSIMD
cannot touch PSUM on real HW):
  PE: matmuls + transposes.
  Act: Exp/Gelu/Ln + PSUM evictions as Copy (in every act table). Each
    Act instruction pays a ~185ns bubble, so softmax Exp is batched
    across heads via multi-score PSUM tiles, and the LN rsqrt is
    exp(-0.5*ln(var+eps)) so it shares the natural_log_exp table with
    softmax's Exp (a table switch costs 1.38us; only Gelu switches).
  DVE: bn_stats/aggr, V copies, AV evict+1/Z (batched, broadcast rz),
    residual adds, reciprocals.
  Pool (gpsimd): LN applies (SBUF-only), memsets.
  SP (sync): all DMA (transfer time is charged to the issuing engine).
"""

import os
import sys

import numpy as np

for _p in ("/opt/trn_rl_repo", "/root/.axon_site/_ro/trn_rl_repo"):
    if os.path.isdir(_p) and _p not in sys.path:
        sys.path.append(_p)

import ml_dtypes  # noqa: E402
import concourse.bass as bass  # noqa: E402
import concourse.mybir as mybir  # noqa: E402
import concourse.tile as tile  # noqa: E402
from concourse import bacc  # noqa: E402
from concourse.bass_utils import run_bass_kernel_spmd  # noqa: E402
from concourse.masks import make_identity  # noqa: E402

F32 = mybir.dt.float32
BF16 = mybir.dt.bfloat16
FP8 = mybir.dt.float8e4
AF = mybir.ActivationFunctionType
ALU = mybir.AluOpType
PM = mybir.MatmulPerfMode

B, C, H, W = 64, 3, 224, 224
P, NP, D, HEADS, DEPTH = 16, 196, 384, 6, 6
DD, DHEADS, DDEPTH = 256, 8, 4
NVIS = 49
DH = D // HEADS          # 64
DDH = DD // DHEADS       # 32
NCORES = 8
BS = B // NCORES         # 8 samples per core
PATCH = C * P * P        # 768
EPS = 1e-5

# encoder token layout: densely packed 50/sample -> 400 tokens,
# tiles of [128, 128, 128, 16]
ET = 50 * BS             # 400
ECH = 4
ESZ = [128, 128, 128, 16]
EOFF = [0, 128, 256, 384]
EKC = D // 128           # 3
EFH = 4 * D              # 1536
# decoder token layout: 196/sample = 2x98 tiles -> 16x[98] tiles
DT = NP * BS             # 1568
DTILES = 2 * BS          # 16
DKC = DD // 128          # 2
DFH = 4 * DD             # 1024
DNCH = 392               # linear N-chunk width for decoder (4 chunks)

# fp8 (e4m3, DoubleRow matmul) scaling constants — fixed powers of two baked
# into the program (exp/evict scale immediates). Weights absmax ~0.1, LN
# outputs ~N(0,1) absmax ~5, so margins to the 448 fp8 max are >4x.
SXE = 16.0               # enc LN1 output fp8 scale
SWE = 1024.0             # enc qkv + out-proj weight fp8 scale
SOE = 8.0                # enc attention-output fp8 scale
SVD = 4.0                # dec V fp8 scale
EKP = 512                # enc contract dim padded for DoubleRow (384 -> 512)
ES_SCALE = 1.0 / (np.sqrt(DH) * (SXE * SWE) ** 2)   # enc softmax exp scale
EV_SCALE = SOE / (SXE * SWE)                        # enc V eviction scale
EO_SCALE = 1.0 / (SOE * SWE)                        # enc out-proj evict scale

_CACHE = {}


def _bf(x):
    return np.ascontiguousarray(np.asarray(x, np.float32).astype(ml_dtypes.bfloat16))


def _f8(x, scale):
    a = np.asarray(x, np.float32) * scale
    amax = np.abs(a).max()
    assert amax < 400.0, f"fp8 overflow risk: scaled absmax {amax}"
    return np.ascontiguousarray(a.astype(ml_dtypes.float8_e4m3))


def _f32(x):
    return np.ascontiguousarray(np.asarray(x, np.float32))



def _build_program():
    nc = bacc.Bacc()

    # ---- DRAM parameters (per core) ----
    xg_t = nc.declare_dram_parameter("xg_t", [PATCH, 50 * BS], BF16, isOutput=False)
    bias_tok = nc.declare_dram_parameter("bias_tok", [ET, D], F32, isOutput=False)
    dpe = nc.declare_dram_parameter("dpe", [DT, DD], F32, isOutput=False)
    wconv_t = nc.declare_dram_parameter("wconv_t", [PATCH, D], BF16, isOutput=False)
    e_qkvw = nc.declare_dram_parameter("e_qkvw", [DEPTH, EKP, 3 * D], FP8, isOutput=False)
    e_ow = nc.declare_dram_parameter("e_ow", [DEPTH, EKP, D], FP8, isOutput=False)
    e_w1 = nc.declare_dram_parameter("e_w1", [DEPTH, D, EFH], BF16, isOutput=False)
    e_w2 = nc.declare_dram_parameter("e_w2", [DEPTH, EFH, D], BF16, isOutput=False)
    d_qkvw = nc.declare_dram_parameter("d_qkvw", [DDEPTH, DD, 3 * DD], BF16, isOutput=False)
    d_ow = nc.declare_dram_parameter("d_ow", [DDEPTH, DD, DD], BF16, isOutput=False)
    d_w1 = nc.declare_dram_parameter("d_w1", [DDEPTH, DD, DFH], BF16, isOutput=False)
    d_w2 = nc.declare_dram_parameter("d_w2", [DDEPTH, DFH, DD], BF16, isOutput=False)
    dew = nc.declare_dram_parameter("dew", [D, DD], BF16, isOutput=False)
    predw = nc.declare_dram_parameter("predw", [DD, P * P * C], BF16, isOutput=False)
    out = nc.declare_dram_parameter("out", [DT, P * P * C], BF16, isOutput=True)

    with tile.TileContext(nc) as tc:
        with (
            tc.tile_pool(name="persist", bufs=1) as persist,
            tc.tile_pool(name="spool", bufs=12) as spool,
        ):
            ident = persist.tile([128, 128], BF16, tag="ident", name="ident")
            make_identity(nc, ident[:])
            ident8 = persist.tile([128, 128], FP8, tag="ident8", name="ident8")
            make_identity(nc, ident8[:])
            eps_t = persist.tile([128, 1], F32, tag="eps", name="eps")
            nc.gpsimd.memset(eps_t[:], EPS)
            lnsx_t = persist.tile([128, 1], F32, tag="lnsx", name="lnsx")
            nc.gpsimd.memset(lnsx_t[:], float(np.log(SXE)))

            def ln_phase(x_list, sizes, emit, base=0, ri_bias=None):
                """Batched LN over the tiles.

                rsqrt computed as exp(-0.5*ln(var+eps)) so the Act engine
                stays on the natural_log_exp table (shared with softmax Exp).
                emit(t, mb_ap, ri_ap) applies the normalize for tile base+t.
                ri_bias=ln(s) folds an fp8 output scale s into ri/mb for free.
                """
                nt = len(x_list)
                rows = max(sizes)
                mvp = spool.tile([rows, nt, 2], F32, tag="mvp", name="mvp", bufs=4)
                if min(sizes) < rows:
                    nc.gpsimd.memset(mvp[:], 1.0)
                for t in range(nt):
                    st6 = spool.tile([rows, 6], F32, tag="st6", name="st6")
                    nc.vector.bn_stats(st6[:sizes[t]], x_list[t][:])
                    nc.vector.bn_aggr(mvp[:sizes[t], t, :], st6[:sizes[t]])
                lv = spool.tile([rows, nt], F32, tag="lv", name="lv", bufs=4)
                nc.scalar.activation(lv[:], mvp[:, :, 1], AF.Ln, bias=eps_t[:rows])
                ri = spool.tile([rows, nt], F32, tag="ri", name="ri", bufs=4)
                if ri_bias is None:
                    nc.scalar.activation(ri[:], lv[:], AF.Exp, scale=-0.5)
                else:
                    nc.scalar.activation(ri[:], lv[:], AF.Exp, scale=-0.5,
                                         bias=ri_bias[:rows])
                mb = spool.tile([rows, nt], F32, tag="mb", name="mb", bufs=4)
                nc.gpsimd.tensor_tensor(mb[:], mvp[:, :, 0], ri[:], ALU.mult)
                for t in range(nt):
                    emit(base + t, mb[:sizes[t], t:t + 1], ri[:sizes[t], t:t + 1])

            def pack_transpose(srcs, dsts, sizes, gsz, pool, tag, maxw=1024,
                               col0=0, evict="act", dt=BF16):
                """dsts[f][:, col0+off_t : ...] = srcs[t][:, 128f:128(f+1)].T

                Groups gsz transposes into one PSUM bank, evicted by a single
                wide Act copy. sizes[t] = partition count of srcs[t].
                """
                nt, nf = len(srcs), len(dsts)
                offs = [sum(sizes[:t]) for t in range(nt + 1)]
                gi = 0
                for f in range(nf):
                    for g0 in range(0, nt, gsz):
                        g = min(gsz, nt - g0)
                        w = offs[g0 + g] - offs[g0]
                        # fp8 PE transposes must write with element step 2 —
                        # give the PSUM tile a stride-2 last dim in that case
                        if dt == FP8:
                            ptw = pool.tile([128, maxw, 2], FP8, tag=tag,
                                            name="ptr")
                            pt = ptw[:, :, 0]
                            idt = ident8
                        else:
                            pt = pool.tile([128, maxw], dt, tag=tag, name="ptr")
                            idt = ident
                        for k in range(g):
                            r = sizes[g0 + k]
                            o = offs[g0 + k] - offs[g0]
                            nc.tensor.transpose(
                                pt[:128, o:o + r],
                                srcs[g0 + k][:, 128 * f:128 * (f + 1)],
                                idt[:r, :r])
                        dst = dsts[f][:, col0 + offs[g0]:col0 + offs[g0 + g]]
                        eng = evict if evict != "alt" else ("act" if gi % 2 == 0
                                                            else "dve")
                        if eng == "act":
                            nc.scalar.activation(dst, pt[:128, :w], AF.Copy)
                        else:
                            nc.vector.tensor_copy(dst, pt[:128, :w])
                        gi += 1

            def ln_apply(dst, src, mb, ri):
                # dst = src * ri - mb, mb = mean*ri  (per-partition affine, Pool)
                nc.gpsimd.tensor_scalar(dst, src, ri, mb, ALU.mult,
                                        ALU.subtract)

            # encoder N-chunking: tiles {0,1} -> cols 0:256, {2,3} -> 256:400,
            # so downstream GEMM chunks pipeline behind the LN/pack of their
            # half via subtile deps
            ECHN = [(0, 256), (256, ET)]

            def enc_ln(emit, xhs, xT, ptrp, evict="act", dt=BF16, ri_bias=None):
                for hb in range(2):
                    ln_phase(x_e[2 * hb:2 * hb + 2], ESZ[2 * hb:2 * hb + 2],
                             emit, base=2 * hb, ri_bias=ri_bias)
                    pack_transpose(xhs[2 * hb:2 * hb + 2], xT,
                                   ESZ[2 * hb:2 * hb + 2], 2, ptrp, "ptr",
                                   maxw=ET, col0=256 * hb, evict=evict, dt=dt)

            # ================= ENCODER =================
            x_e = [persist.tile([ESZ[c], D], F32, tag=f"x_e{c}", name=f"x_e{c}")
                   for c in range(ECH)]
            x_dt = persist.tile([98, DTILES, DD], F32, tag="x_d", name="x_d")
            x_d = [x_dt[:, t, :] for t in range(DTILES)]

            with (
                tc.tile_pool(name="ewpool", bufs=2) as wpool,
                tc.tile_pool(name="eapool", bufs=1) as apool,
                tc.tile_pool(name="epsb", bufs=2, space="PSUM") as psb,
                tc.tile_pool(name="epsS", bufs=2, space="PSUM") as psS,
                tc.tile_pool(name="epsO", bufs=2, space="PSUM") as psO,
                tc.tile_pool(name="eptr", bufs=2, space="PSUM") as ptrp,
            ):
                # patch embed
                xg_sb = apool.tile([128, PATCH // 128, 50 * BS], BF16, tag="xg",
                                   name="xg")
                nc.sync.dma_start(
                    xg_sb[:], xg_t[:].rearrange("(kc p) t -> p kc t", p=128))
                wc_sb = apool.tile([128, PATCH // 128, D], BF16, tag="wc", name="wc")
                nc.gpsimd.dma_start(
                    wc_sb[:], wconv_t[:].rearrange("(kc p) f -> p kc f", p=128))
                bias_sb = [apool.tile([ESZ[c], D], F32, tag=f"btok{c}",
                                      name=f"btok{c}") for c in range(ECH)]
                for c in range(ECH):
                    nc.gpsimd.dma_start(bias_sb[c][:],
                                        bias_tok[EOFF[c]:EOFF[c] + ESZ[c], :])
                for c in range(ECH):
                    ps = psb.tile([ESZ[c], D], F32, tag="big", name="psbig")
                    for kc in range(PATCH // 128):
                        nc.tensor.matmul(ps[:],
                                         xg_sb[:, kc, EOFF[c]:EOFF[c] + ESZ[c]],
                                         wc_sb[:, kc, :],
                                         start=(kc == 0),
                                         stop=(kc == PATCH // 128 - 1))
                    nc.vector.tensor_add(x_e[c][:], bias_sb[c][:], ps[:])

                def enc_layer(i):
                    qkvw_sb = wpool.tile([128, 2, 2, 3 * D], FP8, tag="eqkvw",
                                         name="eqkvw")
                    nc.sync.dma_start(qkvw_sb[:],
                                      e_qkvw[i].rearrange("(pr j p) f -> p pr j f",
                                                          p=128, j=2))
                    ow_sb = wpool.tile([128, 2, 2, D], FP8, tag="eow", name="eow")
                    nc.sync.dma_start(ow_sb[:],
                                      e_ow[i].rearrange("(pr j p) f -> p pr j f",
                                                        p=128, j=2))
                    w1_sb = wpool.tile([128, EKC, EFH], BF16, tag="ew1", name="ew1")
                    nc.sync.dma_start(w1_sb[:],
                                      e_w1[i].rearrange("(kc p) f -> p kc f", p=128))
                    w2_sb = wpool.tile([128, EFH // 128, D], BF16, tag="ew2",
                                       name="ew2")
                    nc.sync.dma_start(w2_sb[:],
                                      e_w2[i].rearrange("(kc p) f -> p kc f", p=128))

                    # LN1 + transpose (fp8, scaled by SXE via the Exp bias;
                    # k-tile 3 is DoubleRow padding — W rows there are zero,
                    # memset keeps the garbage finite)
                    xTb = apool.tile([128, 4, ET], FP8, tag="exT8", name="exT8",
                                     bufs=2)
                    nc.gpsimd.memset(xTb[:, 3, :], 0.0)
                    xT = [xTb[:, f, :] for f in range(EKC)]
                    xhs = [apool.tile([ESZ[c], D], FP8, tag=f"exh{c}", name=f"exh{c}",
                                      bufs=2) for c in range(ECH)]

                    def emit_ln1(t, mb, ri, xhs=xhs):
                        ln_apply(xhs[t][:], x_e[t][:], mb, ri)

                    enc_ln(emit_ln1, xhs, xT, ptrp, dt=FP8, ri_bias=lnsx_t)

                    # QKV + V + attention interleaved per N-chunk: chunk 0
                    # covers samples 0-4, chunk 1 samples 5-7, so attention
                    # overlaps the second chunk's evictions
                    qk = [apool.tile([128, ET], BF16, tag=f"eqk{f}", name=f"eqk{f}", bufs=2)
                          for f in range(6)]
                    qkh = [apool.tile([64, ET], BF16, tag=f"eqkh{f}",
                                      name=f"eqkh{f}", bufs=2) for f in range(6)]
                    v_sb = [apool.tile([50, HEADS * (DH + 1)], BF16, tag=f"ev{s}",
                                       name=f"ev{s}") for s in range(BS)]
                    o_sb = [apool.tile([50, D], FP8, tag=f"eo{s}", name=f"eo{s}")
                            for s in range(BS)]
                    for ci, (c0, c1) in enumerate(ECHN):
                        for fo in range(6):
                            ps = psb.tile([128, 256], F32, tag="big", name="psbig")
                            for pr in range(2):
                                nc.tensor.matmul(
                                    ps[:, 0:c1 - c0],
                                    qkvw_sb[:, pr, :, 128 * fo:128 * (fo + 1)],
                                    xTb[:, 2 * pr:2 * pr + 2, c0:c1],
                                    start=(pr == 0), stop=(pr == 1),
                                    perf_mode=PM.DoubleRow)
                            if ci == 0:
                                nc.vector.tensor_copy(qk[fo][:, c0:c1],
                                                      ps[:, 0:c1 - c0])
                            else:
                                nc.scalar.activation(qk[fo][:, c0:c1],
                                                     ps[:, 0:c1 - c0], AF.Copy)
                            # rebase top half to partition 0 so odd heads'
                            # score matmuls can col-pack into one PSUM bank
                            nc.gpsimd.dma_start(qkh[fo][:, c0:c1],
                                                qk[fo][64:128, c0:c1])
                        for s in range(5 * ci, 5 if ci == 0 else BS):
                            ps = psb.tile([50, D], F32, tag="big", name="psbig")
                            for pr in range(2):
                                nc.tensor.matmul(ps[:],
                                                 xTb[:, 2 * pr:2 * pr + 2,
                                                     50 * s:50 * s + 50],
                                                 qkvw_sb[:, pr, :, 2 * D:3 * D],
                                                 start=(pr == 0), stop=(pr == 1),
                                                 perf_mode=PM.DoubleRow)
                            v3 = v_sb[s][:].rearrange("p (h e) -> p h e", e=DH + 1)
                            if s % 2 == 0:
                                nc.vector.tensor_scalar(
                                    v3[:, :, 0:DH],
                                    ps[:].rearrange("p (h e) -> p h e", e=DH),
                                    EV_SCALE, None, ALU.mult)
                            else:
                                nc.scalar.activation(
                                    v3[:, :, 0:DH],
                                    ps[:].rearrange("p (h e) -> p h e", e=DH),
                                    AF.Copy, scale=EV_SCALE)
                            nc.gpsimd.memset(v3[:, :, DH:DH + 1], 1.0)
                        # attention: all 6 head scores in one PSUM bank -> one
                        # Exp; all 6 AV outputs in one bank -> one scaled
                        # evict. Chunk-0 samples (0-4, cols < 256) start as
                        # soon as chunk 0's q/k/v land, overlapping chunk 1's
                        # QKV evictions.
                        for s in range(5 * ci, 5 if ci == 0 else BS):
                            sps = psS.tile([50, HEADS * 50], F32, tag="S",
                                           name="esS")
                            for h in range(HEADS):
                                kc_h, off = divmod(DH * h, 128)
                                ksrc = qk[3 + kc_h] if off == 0 else qkh[3 + kc_h]
                                qsrc = qk[kc_h] if off == 0 else qkh[kc_h]
                                nc.tensor.matmul(
                                    sps[:, 50 * h:50 * (h + 1)],
                                    ksrc[0:DH, 50 * s:50 * s + 50],
                                    qsrc[0:DH, 50 * s:50 * s + 50],
                                    start=True, stop=True)
                            es = apool.tile([50, HEADS * 50], BF16, tag="ees",
                                            name="ees", bufs=3)
                            nc.scalar.activation(es[:], sps[:], AF.Exp,
                                                 scale=ES_SCALE)
                            ops = psO.tile([50, HEADS, DH + 1], F32, tag="O",
                                           name="eops")
                            for h in range(HEADS):
                                nc.tensor.matmul(
                                    ops[:, h, :], es[:, 50 * h:50 * (h + 1)],
                                    v_sb[s][:, (DH + 1) * h:(DH + 1) * (h + 1)],
                                    start=True, stop=True)
                            rz = spool.tile([50, HEADS, 1], F32, tag="erz",
                                            name="erz")
                            nc.vector.reciprocal(rz[:], ops[:, :, DH:DH + 1])
                            nc.vector.tensor_tensor(
                                o_sb[s][:].rearrange("p (h e) -> p h e", e=DH),
                                ops[:, :, 0:DH],
                                rz[:].to_broadcast([50, HEADS, DH]),
                                ALU.mult)

                    # transpose O (fp8), out-proj, residual
                    oTb = apool.tile([128, 4, ET], FP8, tag="eoT8", name="eoT8",
                                     bufs=2)
                    nc.gpsimd.memset(oTb[:, 3, :], 0.0)
                    oT = [oTb[:, f, :] for f in range(EKC)]
                    pack_transpose(o_sb, oT, [50] * BS, 4, ptrp, "ptr", maxw=ET,
                                   evict="alt", dt=FP8)
                    # LN2 + transpose, FC1+gelu, FC2, residual
                    x2T = [apool.tile([128, ET], BF16, tag=f"ex2T{f}", name=f"ex2T{f}", bufs=2)
                           for f in range(EKC)]
                    xh2s = [apool.tile([ESZ[c], D], BF16, tag=f"exh2{c}",
                                       name=f"exh2{c}", bufs=2) for c in range(ECH)]

                    def emit_ln2(t, mb, ri, xh2s=xh2s):
                        ln_apply(xh2s[t][:], x_e[t][:], mb, ri)

                    # out-proj+residual per half interleaved with ln2 of that
                    # half: ln2's first half only needs residuals of chunks 0-1
                    for hb in range(2):
                        for c in (2 * hb, 2 * hb + 1):
                            ps = psb.tile([ESZ[c], D], F32, tag="big", name="psbig")
                            for pr in range(2):
                                nc.tensor.matmul(ps[:],
                                                 oTb[:, 2 * pr:2 * pr + 2,
                                                     EOFF[c]:EOFF[c] + ESZ[c]],
                                                 ow_sb[:, pr, :, :],
                                                 start=(pr == 0), stop=(pr == 1),
                                                 perf_mode=PM.DoubleRow)
                            nc.vector.scalar_tensor_tensor(
                                x_e[c][:], ps[:], EO_SCALE, x_e[c][:],
                                ALU.mult, ALU.add)
                        ln_phase(x_e[2 * hb:2 * hb + 2], ESZ[2 * hb:2 * hb + 2],
                                 emit_ln2, base=2 * hb)
                        pack_transpose(xh2s[2 * hb:2 * hb + 2], x2T,
                                       ESZ[2 * hb:2 * hb + 2], 2, ptrp, "ptr",
                                       maxw=ET, col0=256 * hb, evict="dve")
                    hsb = [apool.tile([128, ET], BF16, tag=f"eh{f}", name=f"eh{f}", bufs=2)
                           for f in range(EFH // 128)]
                    for fo in range(EFH // 128):
                        ps = psb.tile([128, ET], F32, tag="big", name="psbig")
                        for c0, c1 in ECHN:
                            for kc in range(EKC):
                                nc.tensor.matmul(
                                    ps[:, c0:c1],
                                    w1_sb[:, kc, 128 * fo:128 * (fo + 1)],
                                    x2T[kc][:, c0:c1], start=(kc == 0),
                                    stop=(kc == EKC - 1))
                        nc.scalar.activation(hsb[fo][:], ps[:], AF.Gelu)
                    for c in range(ECH):
                        ps = psb.tile([ESZ[c], D], F32, tag="big", name="psbig")
                        for kc in range(EFH // 128):
                            nc.tensor.matmul(ps[:],
                                             hsb[kc][:, EOFF[c]:EOFF[c] + ESZ[c]],
                                             w2_sb[:, kc, :],
                                             start=(kc == 0),
                                             stop=(kc == EFH // 128 - 1))
                        nc.vector.tensor_add(x_e[c][:], x_e[c][:], ps[:])

                for i in range(DEPTH):
                    enc_layer(i)

                # ==== encoder final LN + dec-embed -> decoder init ====
                for t in range(DTILES):
                    nc.gpsimd.dma_start(x_d[t], dpe[98 * t:98 * (t + 1), :])
                dew_sb = apool.tile([128, EKC, DD], BF16, tag="dew", name="dew")
                nc.sync.dma_start(dew_sb[:],
                                  dew[:].rearrange("(kc p) f -> p kc f", p=128))
                xfT = [apool.tile([128, ET], BF16, tag=f"exT{f}", name=f"exT{f}", bufs=2)
                       for f in range(EKC)]
                xhfs = [apool.tile([ESZ[c], D], BF16, tag=f"exh{c}", name=f"exh{c}",
                                   bufs=2) for c in range(ECH)]

                def emit_lnf(t, mb, ri):
                    ln_apply(xhfs[t][:], x_e[t][:], mb, ri)

                def dec_embed(s):
                    ps = psb.tile([NVIS, DD], F32, tag="big", name="psbig")
                    for kc in range(EKC):
                        nc.tensor.matmul(ps[:],
                                         xfT[kc][:, 50 * s + 1:50 * s + 1 + NVIS],
                                         dew_sb[:, kc, :],
                                         start=(kc == 0), stop=(kc == EKC - 1))
                    nc.vector.tensor_add(x_dt[0:NVIS, 2 * s, :],
                                         x_dt[0:NVIS, 2 * s, :], ps[:])

                # interleave dec-embed with the final LN halves (chunk 0
                # covers samples 0-4)
                for hb in range(2):
                    ln_phase(x_e[2 * hb:2 * hb + 2], ESZ[2 * hb:2 * hb + 2],
                             emit_lnf, base=2 * hb)
                    pack_transpose(xhfs[2 * hb:2 * hb + 2], xfT,
                                   ESZ[2 * hb:2 * hb + 2], 2, ptrp, "ptr",
                                   maxw=ET, col0=256 * hb)
                    for s in range(5 * hb, 5 if hb == 0 else BS):
                        dec_embed(s)

            # ================= DECODER =================
            with (
                tc.tile_pool(name="dwpool", bufs=2) as wpool,
                tc.tile_pool(name="dapool", bufs=1) as apool,
                tc.tile_pool(name="dpsb", bufs=2, space="PSUM") as psb,
                tc.tile_pool(name="dpsq", bufs=2, space="PSUM") as psq,
                tc.tile_pool(name="dpsm", bufs=2, space="PSUM") as psm,
            ):
                def dec_ln(emit, xhs, xT, pools=None):
                    # halves, so downstream per-tile work pipelines with the
                    # second half's stats
                    for hb in range(2):
                        ln_phase(x_d[8 * hb:8 * hb + 8], [98] * 8, emit,
                                 base=8 * hb)
                        pack_transpose(xhs[8 * hb:8 * hb + 8], xT, [98] * 8, 4,
                                       psm, "sm", maxw=784, col0=784 * hb,
                                       evict="alt")

                def dec_layer(i):
                    qkvw_sb = wpool.tile([128, DKC, 3 * DD], BF16, tag="dqkvw",
                                         name="dqkvw")
                    nc.sync.dma_start(qkvw_sb[:],
                                      d_qkvw[i].rearrange("(kc p) f -> p kc f", p=128))
                    ow_sb = wpool.tile([128, DKC, DD], BF16, tag="dow", name="dow")
                    nc.sync.dma_start(ow_sb[:],
                                      d_ow[i].rearrange("(kc p) f -> p kc f", p=128))
                    w1_sb = wpool.tile([128, DKC, DFH], BF16, tag="dw1", name="dw1")
                    nc.sync.dma_start(w1_sb[:],
                                      d_w1[i].rearrange("(kc p) f -> p kc f", p=128))
                    w2_sb = wpool.tile([128, DFH // 128, DD], BF16, tag="dw2",
                                       name="dw2")
                    nc.sync.dma_start(w2_sb[:],
                                      d_w2[i].rearrange("(kc p) f -> p kc f", p=128))

                    xT = [apool.tile([128, DT], BF16, tag=f"dxT{f}", name=f"dxT{f}", bufs=2)
                          for f in range(DKC)]
                    xhs = [apool.tile([98, DD], BF16, tag=f"dxh{t}", name=f"dxh{t}")
                           for t in range(DTILES)]

                    def emit_dln1(t, mb, ri, xhs=xhs):
                        ln_apply(xhs[t][:], x_d[t], mb, ri)

                    dec_ln(emit_dln1, xhs, xT)

                    # QKV + V + attention interleaved per no-chunk so the
                    # attention of chunk n's samples overlaps chunk n+1's
                    # evictions instead of waiting for all of them
                    qk = [apool.tile([128, DT], BF16, tag=f"dqk{f}", name=f"dqk{f}", bufs=2)
                          for f in range(4)]
                    v_bt = apool.tile([98, DTILES, DHEADS * (DDH + 1)], FP8,
                                      tag="dv", name="dv")
                    v_sb = [v_bt[:, t, :] for t in range(DTILES)]
                    o_sb = [apool.tile([98, DD], BF16, tag=f"do{t}", name=f"do{t}")
                            for t in range(DTILES)]
                    def dec_att(s):
                            # per head-quad (2 head-pairs): AV outputs gather
                            # in one PSUM bank so the rz normalize needs one
                            # reciprocal + 2 scaled evicts per quad
                            for hq in range(2):
                                ops = psm.tile([98, 4, 2, DDH + 1], F32,
                                               tag="sm", name="dops")
                                for hp2 in range(2):
                                    hp = 2 * hq + hp2
                                    for hh in range(2):
                                        h = 2 * hp + hh
                                        kc_h, off = divmod(DDH * h, 128)
                                        sps = psq.tile([128, 2, 512], F32,
                                                       tag="q", name="dsq")
                                        for j in range(2):
                                            # each key-block in its own bank at
                                            # byte offset 0: PE matmuls at a
                                            # PSUM byte offset need base-0
                                            # operands
                                            nc.tensor.matmul(
                                                sps[:98, j, 0:NP],
                                                qk[2 + kc_h][off:off + DDH,
                                                             NP * s + 98 * j:NP * s + 98 * (j + 1)],
                                                qk[kc_h][off:off + DDH,
                                                         NP * s:NP * (s + 1)],
                                                start=True, stop=True,
                                                tile_position=(off, 0))
                                        # free dim padded 196 -> 208: fp8
                                        # DoubleRow Ldweights needs a
                                        # 16-multiple k-tile step
                                        es = apool.tile([98, 2, 208], FP8,
                                                        tag="des", name="des",
                                                        bufs=6)
                                        nc.scalar.activation(es[:, :, 0:NP],
                                                             sps[:98, :, 0:NP],
                                                             AF.Exp)
                                        for qc in range(2):
                                            # fp8 DoubleRow: both key blocks in
                                            # one matmul; the SVD scale on V
                                            # cancels via the ones column
                                            nc.tensor.matmul(
                                                ops[:, 2 * hp2 + hh, qc, :],
                                                es[:, :, 98 * qc:98 * (qc + 1)],
                                                v_bt[:, 2 * s:2 * s + 2,
                                                     (DDH + 1) * h:(DDH + 1) * (h + 1)],
                                                start=True, stop=True,
                                                perf_mode=PM.DoubleRow)
                                rz = spool.tile([98, 4, 2, 1], F32, tag="drz",
                                                name="drz")
                                nc.vector.reciprocal(rz[:], ops[:, :, :, DDH:DDH + 1])
                                for qc in range(2):
                                    nc.vector.tensor_tensor(
                                        o_sb[2 * s + qc][:, DDH * 4 * hq:
                                                         DDH * 4 * (hq + 1)]
                                        .rearrange("p (hh e) -> p hh e", e=DDH),
                                        ops[:, :, qc, 0:DDH],
                                        rz[:, :, qc, :].to_broadcast([98, 4, DDH]),
                                        ALU.mult)

                    oT = [apool.tile([128, DT], BF16, tag=f"doT{f}", name=f"doT{f}", bufs=2)
                          for f in range(DKC)]

                    def opack_outproj(hb):
                        pack_transpose(o_sb[8 * hb:8 * hb + 8], oT, [98] * 8, 8,
                                       psm, "sm", maxw=784, col0=784 * hb,
                                       evict="alt")
                        for u in range(4 * hb, 4 * hb + 4):
                            ps = psb.tile([98, 2, DD], F32, tag="big", name="psbig")
                            for half in range(2):
                                t = 2 * u + half
                                for kc in range(DKC):
                                    nc.tensor.matmul(
                                        ps[:, half, :], oT[kc][:, 98 * t:98 * (t + 1)],
                                        ow_sb[:, kc, :],
                                        start=(kc == 0), stop=(kc == DKC - 1))
                            nc.vector.tensor_add(x_dt[:, 2 * u:2 * u + 2, :],
                                                 x_dt[:, 2 * u:2 * u + 2, :], ps[:])

                    x2T = [apool.tile([128, DT], BF16, tag=f"dx2T{f}",
                                      name=f"dx2T{f}") for f in range(DKC)]
                    xh2s = [apool.tile([98, DD], BF16, tag=f"dxh2{t}",
                                       name=f"dxh2{t}") for t in range(DTILES)]

                    def emit_dln2(t, mb, ri, xh2s=xh2s):
                        ln_apply(xh2s[t][:], x_d[t], mb, ri)

                    def ln2_stats(hb):
                        ln_phase(x_d[8 * hb:8 * hb + 8], [98] * 8, emit_dln2,
                                 base=8 * hb)

                    def ln2_pack(hb):
                        pack_transpose(xh2s[8 * hb:8 * hb + 8], x2T, [98] * 8, 4,
                                       psm, "sm", maxw=784, col0=784 * hb,
                                       evict="dve")

                    # FC1: pairs of hidden-feature rows into one 2-bank tile
                    # -> single Gelu over both banks
                    hsb = [apool.tile([128, 2, DT], BF16, tag=f"dh{f2}",
                                      name=f"dh{f2}") for f2 in range(DFH // 256)]
                    hview = lambda kc: hsb[kc // 2][:, kc % 2, :]

                    def fc1_no(no):
                        for f2 in range(DFH // 256):
                            ps = psq.tile([128, 2, 512], F32, tag="q",
                                          name="dfc1")
                            for j in range(2):
                                fo = 2 * f2 + j
                                for kc in range(DKC):
                                    nc.tensor.matmul(
                                        ps[:, j, 0:DNCH],
                                        w1_sb[:, kc, 128 * fo:128 * (fo + 1)],
                                        x2T[kc][:, DNCH * no:DNCH * (no + 1)],
                                        start=(kc == 0), stop=(kc == DKC - 1))
                            nc.scalar.activation(
                                hsb[f2][:, :, DNCH * no:DNCH * (no + 1)],
                                ps[:, :, 0:DNCH], AF.Gelu)

                    def fc2_u(u):
                        ps = psb.tile([98, 2, DD], F32, tag="big", name="psbig")
                        for half in range(2):
                            t = 2 * u + half
                            for kc in range(DFH // 128):
                                nc.tensor.matmul(
                                    ps[:, half, :], hview(kc)[:, 98 * t:98 * (t + 1)],
                                    w2_sb[:, kc, :],
                                    start=(kc == 0),
                                    stop=(kc == DFH // 128 - 1))
                        nc.vector.tensor_add(x_dt[:, 2 * u:2 * u + 2, :],
                                             x_dt[:, 2 * u:2 * u + 2, :], ps[:])

                    for no in range(4):
                        for fo in range(4):
                            ps = psb.tile([128, DNCH], F32, tag="big", name="psbig")
                            for kc in range(DKC):
                                nc.tensor.matmul(
                                    ps[:], qkvw_sb[:, kc, 128 * fo:128 * (fo + 1)],
                                    xT[kc][:, DNCH * no:DNCH * (no + 1)],
                                    start=(kc == 0), stop=(kc == DKC - 1))
                            nc.vector.tensor_copy(
                                qk[fo][:, DNCH * no:DNCH * (no + 1)], ps[:])
                        for u in range(2 * no, 2 * no + 2):
                            ps = psb.tile([98, 2, DD], F32, tag="big", name="psbig")
                            for half in range(2):
                                t = 2 * u + half
                                for kc in range(DKC):
                                    nc.tensor.matmul(
                                        ps[:, half, :], xT[kc][:, 98 * t:98 * (t + 1)],
                                        qkvw_sb[:, kc, 2 * DD:3 * DD],
                                        start=(kc == 0), stop=(kc == DKC - 1))
                            v3 = v_bt[:, 2 * u:2 * u + 2, :].rearrange(
                                "p t (h e) -> p t h e", e=DDH + 1)
                            nc.vector.tensor_scalar(
                                v3[:, :, :, 0:DDH],
                                ps[:].rearrange("p t (h e) -> p t h e", e=DDH),
                                SVD, None, ALU.mult)
                            nc.gpsimd.memset(v3[:, :, :, DDH:DDH + 1], SVD)
                        # attention: per (sample, head-pair) scores per key-
                        # block in its own bank -> Exp per hh; AV quad in one
                        # bank -> 2 scaled evicts. The first half's out-proj
                        # and LN2 slide under the second half's Act-bound exp
                        # window.
                        dec_att(2 * no)
                        if no == 2:
                            opack_outproj(0)
                        if no == 3:
                            ln2_stats(0)
                        dec_att(2 * no + 1)
                        if no == 3:
                            ln2_pack(0)

                    fc1_no(0)
                    fc1_no(1)
                    opack_outproj(1)
                    fc2_u(0)
                    fc2_u(1)
                    ln2_stats(1)
                    fc2_u(2)
                    fc2_u(3)
                    ln2_pack(1)
                    fc1_no(2)
                    fc1_no(3)
                    for u in (4, 5, 6, 7):
                        fc2_u(u)

                for i in range(DDEPTH):
                    dec_layer(i)

                # ======== final LN + pred head ========
                pw_sb = apool.tile([128, DKC, P * P * C], BF16, tag="pw", name="pw")
                nc.sync.dma_start(pw_sb[:],
                                  predw[:].rearrange("(kc p) f -> p kc f", p=128))
                xdT = [apool.tile([128, DT], BF16, tag=f"dxT{f}", name=f"dxT{f}", bufs=2)
                       for f in range(DKC)]
                xhds = [apool.tile([98, DD], BF16, tag=f"dxh{t}", name=f"dxh{t}")
                        for t in range(DTILES)]

                def emit_dlnf(t, mb, ri):
                    ln_apply(xhds[t][:], x_d[t], mb, ri)

                def pred_tile(t):
                    osb = apool.tile([98, P * P * C], BF16, tag="outsb", name="outsb",
                                     bufs=2)
                    for no in range(2):
                        ps = psb.tile([98, 384], F32, tag="big", name="psbig")
                        for kc in range(DKC):
                            nc.tensor.matmul(ps[:], xdT[kc][:, 98 * t:98 * (t + 1)],
                                             pw_sb[:, kc, 384 * no:384 * (no + 1)],
                                             start=(kc == 0), stop=(kc == DKC - 1))
                        if no == 0:
                            nc.vector.tensor_copy(osb[:, 0:384], ps[:])
                        else:
                            nc.scalar.activation(osb[:, 384:768], ps[:], AF.Copy)
                    nc.sync.dma_start(out[98 * t:98 * (t + 1), :], osb[:])

                # interleave pred head of half hb with the final LN of the
                # next half
                for hb in range(2):
                    ln_phase(x_d[8 * hb:8 * hb + 8], [98] * 8, emit_dlnf,
                             base=8 * hb)
                    pack_transpose(xhds[8 * hb:8 * hb + 8], xdT, [98] * 8, 8,
                                   psm, "sm", maxw=784, col0=784 * hb,
                                   evict="alt")
                    for t in range(8 * hb, 8 * hb + 8):
                        pred_tile(t)

    # Pre-place activation-table loads with a restricted table list so the
    # chooser picks natural_log_exp_and_others (serves both the LN-rsqrt's
    # Ln/Exp and softmax's Exp) instead of single-function tables. IDs stay
    # canonical act_info.json indices; finalize's automatic pass then finds
    # every activation already covered and inserts nothing.
    from concourse.hw_specs import get_activation_tables
    import bass_rust as _br
    _tabs = list(get_activation_tables(nc.m.arch).items())
    _keep = {"natural_log_exp_and_others", "gelu_and_others"}
    _tabs = [(k, (v if k in _keep else set())) for k, v in _tabs]
    _br.insert_act_table_loads(nc, _tabs)

    nc.finalize()
    return nc


def _host_prep(inputs):
    ui = np.asarray(inputs["unmasked_idx"])
    mi = np.asarray(inputs["masked_idx"])
    perm = np.concatenate([ui, mi], axis=1)               # [B, 196]
    x = _f32(inputs["x"])
    patches = x.reshape(B, C, 14, P, 14, P).transpose(0, 2, 4, 1, 3, 5) \
               .reshape(B, NP, PATCH)
    bi = np.arange(B)[:, None]
    xg = patches[bi, ui]                                  # [B, 49, 768]
    xg50 = np.concatenate([np.zeros((B, 1, PATCH), np.float32), xg], axis=1)

    pe = _f32(inputs["pos_embed"])
    bias_tok = np.zeros((B, 50, D), np.float32)
    bias_tok[:, 0] = _f32(inputs["cls_token"]) + pe[0]
    bias_tok[:, 1:50] = _f32(inputs["conv_b"]) + pe[ui + 1]

    # decoder positional stream in permuted order, with mask_token and
    # dec-embed bias folded in
    dde = _f32(inputs["dec_pos_embed"])
    bde = _f32(inputs["dec_embed_b"]) + _f32(inputs["dec_embed_w"]) @ _f32(inputs["enc_norm_b"])
    dpe = dde[perm + 1].copy()                            # [B, 196, 256]
    dpe[:, NVIS:] += _f32(inputs["mask_token"])
    dpe[:, :NVIS] += bde

    def fold(pfx, depth, d, dh, prescale_q=True):
        qkvw_t = np.empty((depth, d, 3 * d), np.float32)
        ow_t = np.empty((depth, d, d), np.float32)
        w1_t = np.empty((depth, d, 4 * d), np.float32)
        w2_t = np.empty((depth, 4 * d, d), np.float32)
        for i in range(depth):
            Wq = _f32(inputs[f"{pfx}_qkv_w"][i]) * _f32(inputs[f"{pfx}_ln1_w"][i])[None, :]
            if prescale_q:
                Wq[:d] /= np.sqrt(dh)
            qkvw_t[i] = Wq.T
            ow_t[i] = _f32(inputs[f"{pfx}_out_w"][i]).T
            W1 = _f32(inputs[f"{pfx}_fc1_w"][i]) * _f32(inputs[f"{pfx}_ln2_w"][i])[None, :]
            w1_t[i] = W1.T
            w2_t[i] = _f32(inputs[f"{pfx}_fc2_w"][i]).T
        return qkvw_t, ow_t, w1_t, w2_t

    # enc q/k prescale (1/sqrt(dh)) folds into the softmax Exp scale instead
    # so W_q and W_k share one fp8 scale
    e_qkvw, e_ow, e_w1, e_w2 = fold("enc", DEPTH, D, DH, prescale_q=False)
    d_qkvw, d_ow, d_w1, d_w2 = fold("dec", DDEPTH, DD, DDH)
    # pad enc contract dim 384 -> 512 with zero rows for fp8 DoubleRow pairs
    eq8 = np.zeros((DEPTH, EKP, 3 * D), np.float32)
    eq8[:, :D] = e_qkvw
    eo8 = np.zeros((DEPTH, EKP, D), np.float32)
    eo8[:, :D] = e_ow

    # biases are structurally zero in this model (see spec fills); the LN
    # biases fold into the arrays above. Guard so silent wrongness is
    # impossible if that ever changes.
    for k in ("conv_b", "enc_qkv_b", "enc_out_b", "enc_fc1_b", "enc_fc2_b",
              "dec_qkv_b", "dec_out_b", "dec_fc1_b", "dec_fc2_b",
              "enc_ln1_b", "enc_ln2_b", "dec_ln1_b", "dec_ln2_b",
              "enc_norm_b", "dec_norm_b", "dec_embed_b", "pred_b"):
        if k in ("conv_b",):  # folded into bias_tok already
            continue
        assert np.max(np.abs(_f32(inputs[k]))) == 0.0, f"nonzero bias {k} unsupported"

    dew = (_f32(inputs["dec_embed_w"]) * _f32(inputs["enc_norm_w"])[None, :]).T
    predw = (_f32(inputs["pred_w"]) * _f32(inputs["dec_norm_w"])[None, :]).T
    bp = _f32(inputs["pred_b"]) + _f32(inputs["pred_w"]) @ _f32(inputs["dec_norm_b"])
    wconv_t = _f32(inputs["conv_w"]).reshape(D, PATCH).T

    shared = {
        "wconv_t": _bf(wconv_t),
        "e_qkvw": _f8(eq8, SWE), "e_ow": _f8(eo8, SWE),
        "e_w1": _bf(e_w1), "e_w2": _bf(e_w2),
        "d_qkvw": _bf(d_qkvw), "d_ow": _bf(d_ow),
        "d_w1": _bf(d_w1), "d_w2": _bf(d_w2),
        "dew": _bf(dew), "predw": _bf(predw),
    }
    in_maps = []
    for c in range(NCORES):
        sl = slice(c * BS, (c + 1) * BS)
        m = dict(shared)
        m["xg_t"] = _bf(xg50[sl].reshape(BS * 50, PATCH).T)
        m["bias_tok"] = _f32(bias_tok[sl].reshape(ET, D))
        m["dpe"] = _f32(dpe[sl].reshape(DT, DD))
        in_maps.append(m)
    return in_maps, perm, bp


def kernel(**inputs):
    if "nc" not in _CACHE:
        _CACHE["nc"] = _build_program()
    nc = _CACHE["nc"]
    in_maps, perm, bp = _host_prep(inputs)
    res = run_bass_kernel_spmd(nc, in_maps, list(range(NCORES)))
    _CACHE["last_res"] = res
    dev = np.stack([np.asarray(res.results[c]["out"], np.float32).reshape(BS, NP, P * P * C)
                    for c in range(NCORES)]).reshape(B, NP, P * P * C)
    out = np.empty((B, NP, P * P * C), np.float32)
    out[np.arange(B)[:, None], perm] = dev + bp
    return out

